# revision 14
# baseline (speedup 1.0000x reference)
"""T5-style attention layer (B=4, S=2048, D=1024, H=16, DK=64) on 8 trn2 cores.

Sharding: batch (4) x head-group (2 groups of 8 heads). Core c -> batch c//2,
head-group c%2. Each core computes its batch's attention output restricted to
its 8 heads, projected through its Wo row-slice -> partial [S, D] output.
Host sums the two head-group partials per batch (the "all-reduce").

v2 pipeline (matmuls bf16, fp32 PSUM):
  The ScalarE exp stream is the hard floor (~2.3us per k-tile, 128 k-tile
  instances per core); everything else is scheduled into its shadow.
  - Scores S^T[k,q] per (head-pair m, 1024-q chunk): two [128,2,512] PSUM
    tiles (one per head), row-split matmul pairs. exp on ScalarE reads each
    head's tile; the two heads alternate as natural double buffering.
  - T5 relative-position bias: multiplicative exp(bias) Toeplitz patterns
    applied by DVE (bf16, 4 elem/cycle) AFTER exp on near-diagonal tiles;
    far-saturated tiles fold the per-head constant into the exp bias operand.
  - AV: O^T[dk+1, q] += [V|1].T @ at, one k-tile behind scores; row 64 is
    the softmax denominator. Normalize: reciprocal_approx_fast (DVE) +
    GpSimd partition_broadcast + DVE mul.
  - Q/K/V projections and the output projection are emitted as "background
    packs" interleaved into the k-tile stream so the PE works under the
    ScalarE shadow; only the second half of the output projection trails.
  PSUM: 2 score tiles (2 banks each) rotating in a shared pool (also used
  by projection/output packs) + 4 AV accumulators = 8 banks exactly.
"""

import math
from collections import deque

import ml_dtypes
import numpy as np

import concourse.bass as bass
import concourse.mybir as mybir
import concourse.tile as tile
from concourse import bacc
from concourse.bass_utils import run_bass_kernel_spmd

F32 = mybir.dt.float32
BF16 = mybir.dt.bfloat16
MMDT = BF16
MMNP = ml_dtypes.bfloat16
AF = mybir.ActivationFunctionType

B, S, D, H, DK = 4, 2048, 1024, 16, 64
HG = 8  # heads per core
HDG = HG * DK  # 512
QC = 512  # q chunk width
NKT = S // 128  # 16 k tiles
DBASES = [-128, 0, 128, 256, 384, 512]  # near-band k0-q0 alignments

_NC_CACHE = {}


def _build_nc():
    nc = bacc.Bacc(None, target_bir_lowering=False, debug=False)
    xT = nc.dram_tensor("xT", [D, S], MMDT, kind="ExternalInput")
    wq = nc.dram_tensor("wq", [D, HDG], MMDT, kind="ExternalInput")
    wk = nc.dram_tensor("wk", [D, HDG], MMDT, kind="ExternalInput")
    wv = nc.dram_tensor("wv", [D, HDG], MMDT, kind="ExternalInput")
    wo = nc.dram_tensor("wo", [HDG, D], MMDT, kind="ExternalInput")
    pat = nc.dram_tensor("pat", [HG, len(DBASES) + 2, 128, QC], BF16, kind="ExternalInput")
    cst = nc.dram_tensor("cst", [128, 2 * HG], F32, kind="ExternalInput")
    outd = nc.dram_tensor("out", [S, D], F32, kind="ExternalOutput")

    with tile.TileContext(nc) as tc:
        with tc.tile_pool(name="persist", bufs=1) as persist, tc.tile_pool(
            name="xp", bufs=4
        ) as xpool, tc.tile_pool(name="patp", bufs=2) as patp, tc.tile_pool(
            name="attnp", bufs=4
        ) as attnp, tc.tile_pool(name="rp", bufs=2) as rp, tc.tile_pool(
            name="obp", bufs=2
        ) as obp, tc.tile_pool(name="ps", bufs=2, space="PSUM") as pspool, tc.tile_pool(
            name="ps_o", bufs=4, space="PSUM"
        ) as opool:
            qt = persist.tile([128, 4, S], MMDT, tag="qt")
            kt = persist.tile([128, 4, S], MMDT, tag="kt")
            vt = persist.tile([128, NKT, HG, DK + 1], MMDT, tag="vt")
            csts = persist.tile([128, 2 * HG], F32, tag="csts")
            ot = persist.tile([128, 4, S], MMDT, tag="ot")
            wos = persist.tile([128, 4, D], MMDT, tag="wos")
            wqs = persist.tile([128, 8, HDG], MMDT, tag="wqs")
            wks = persist.tile([128, 8, HDG], MMDT, tag="wks")
            wvs = persist.tile([128, 8, HDG], MMDT, tag="wvs")

            # ---- initial DMAs (ordered so the first QK pack unblocks asap) ----
            nc.sync.dma_start(out=wqs, in_=wq.rearrange("(dc p) n -> p dc n", p=128))
            xqs = {}

            def dma_xq(m, sc):
                t = xpool.tile([128, 8, QC], MMDT, tag="xq")
                nc.sync.dma_start(
                    out=t,
                    in_=xT[:, sc * QC : (sc + 1) * QC].rearrange(
                        "(dc p) s -> p dc s", p=128
                    ),
                )
                xqs[(m, sc)] = t

            dma_xq(0, 0)
            nc.sync.dma_start(out=wks, in_=wk.rearrange("(dc p) n -> p dc n", p=128))
            dma_xq(0, 1)
            nc.sync.dma_start(out=wvs, in_=wv.rearrange("(dc p) n -> p dc n", p=128))
            nc.sync.dma_start(out=csts, in_=cst[:, :])
            nc.sync.dma_start(out=wos, in_=wo.rearrange("(m p) n -> p m n", p=128))
            dma_xq(0, 2)
            dma_xq(0, 3)
            paths = {}

            def dma_path(m):
                t = patp.tile([128, 2, len(DBASES) + 2, QC], BF16, tag="pth")
                nc.sync.dma_start(
                    out=t,
                    in_=pat[2 * m : 2 * m + 2].rearrange("h j p c -> p h j c"),
                )
                paths[m] = t

            dma_path(0)

            # ---- background pack emitters ----
            # Each "half" allocates ONE tile from the shared PSUM pool.  In
            # the steady state the pool alternates between the two score
            # tiles (h0/h1), so background halves must be emitted in PAIRS
            # to preserve slot parity — a lone insertion permanently
            # misaligns the scores->EXP WAR chain and costs ~0.6us of
            # ScalarE idle per k-tile afterwards.
            def q_half(m, sc):
                def emit():
                    ps = pspool.tile([128, 2, QC], F32, tag="sps")
                    xq = xqs[(m, sc)]
                    for dc in range(8):
                        nc.tensor.matmul(
                            ps[:, 0, :],
                            wqs[:, dc, m * 128 : (m + 1) * 128],
                            xq[:, dc, :],
                            start=(dc == 0),
                            stop=(dc == 7),
                        )
                    nc.vector.tensor_copy(
                        qt[:, m, sc * QC : (sc + 1) * QC], ps[:, 0, :]
                    )

                return emit

            def k_half(m, sc):
                def emit():
                    ps = pspool.tile([128, 2, QC], F32, tag="sps")
                    xq = xqs[(m, sc)]
                    for dc in range(8):
                        nc.tensor.matmul(
                            ps[:, 0, :],
                            wks[:, dc, m * 128 : (m + 1) * 128],
                            xq[:, dc, :],
                            start=(dc == 0),
                            stop=(dc == 7),
                        )
                    nc.vector.tensor_copy(
                        kt[:, m, sc * QC : (sc + 1) * QC], ps[:, 0, :]
                    )

                return emit

            def v_half(sc, st):
                def emit():
                    ps = pspool.tile([128, 2, QC], F32, tag="sps")
                    xq = xqs[(0, sc)]
                    for dc in range(8):
                        nc.tensor.matmul(
                            ps[:, 0, :],
                            xq[:, dc, st * 128 : (st + 1) * 128],
                            wvs[:, dc, :],
                            start=(dc == 0),
                            stop=(dc == 7),
                        )
                    nc.vector.tensor_copy(
                        vt[:, sc * 4 + st, :, 0:DK],
                        ps[:, 0, :].rearrange("p (h d) -> p h d", h=HG),
                    )
                    # ones column for the softmax denominator, written by the
                    # same engine as the V copy so ordering vs AV is tracked
                    nc.vector.memset(vt[:, sc * 4 + st, :, DK : DK + 1], 1.0)

                return emit

            def out_pack(st_g):
                def emit():
                    ps = pspool.tile([128, 2, QC], F32, tag="sps")
                    for nck in range(2):
                        for m2 in range(4):
                            nc.tensor.matmul(
                                ps[:, nck, :],
                                ot[:, m2, st_g * 128 : (st_g + 1) * 128],
                                wos[:, m2, nck * 512 : (nck + 1) * 512],
                                start=(m2 == 0),
                                stop=(m2 == 3),
                            )
                    ob = obp.tile([128, 2, QC], F32, tag="ob")
                    nc.vector.tensor_copy(ob, ps)
                    nc.sync.dma_start(
                        out=outd[st_g * 128 : (st_g + 1) * 128, :].rearrange(
                            "p (a b) -> p a b", a=2
                        ),
                        in_=ob,
                    )

                return emit

            def dma_pack(fn, *args):
                def emit():
                    fn(*args)

                return emit

            # ---- attention window ----
            def window(m, qcp, bg):
                bgq = deque(sorted(bg, key=lambda kv: kv[0]))
                path = paths[m]
                o_pss = [
                    opool.tile([DK + 1, QC], F32, tag="ops", name=f"o{i}")
                    for i in range(4)
                ]
                # Emission order per k-tile keeps ScalarE gapless:
                # [pack pair?][scores h0][AV h0(prev)][scores h1][AV h1(prev)]
                # then EXP h0 + bias mults, EXP h1 + bias mults.  scores-h0 of
                # tile t+1 only WARs on EXP-h0(t), which ends one EXP before
                # the ACT queue frees, so the next EXP's input is always ready.
                def emit_scores(sp, hh, kti):
                    for qc2 in range(2):
                        qc = qcp * 2 + qc2
                        nc.tensor.matmul(
                            sp[:, qc2, :],
                            kt[
                                hh * 64 : (hh + 1) * 64,
                                m,
                                kti * 128 : (kti + 1) * 128,
                            ],
                            qt[
                                hh * 64 : (hh + 1) * 64,
                                m,
                                qc * QC : (qc + 1) * QC,
                            ],
                            start=True,
                            stop=True,
                        )

                def emit_av(hh, at_prev, pkti):
                    for qc2 in range(2):
                        nc.tensor.matmul(
                            o_pss[hh * 2 + qc2],
                            vt[:, pkti, 2 * m + hh, :],
                            at_prev[:, qc2, :],
                            start=(pkti == 0),
                            stop=(pkti == NKT - 1),
                        )

                def emit_exp(sp, hh, kti, d0, uniform):
                    h = 2 * m + hh
                    at_t = attnp.tile([128, 2, QC], MMDT, tag="at")
                    if uniform:
                        col = 2 * h + (0 if d0 <= -256 else 1)
                        nc.scalar.activation(
                            at_t, sp, AF.Exp, bias=csts[:, col : col + 1]
                        )
                    else:
                        raw = attnp.tile([128, 2, QC], MMDT, tag="raw")
                        nc.scalar.activation(raw, sp, AF.Exp)
                        for qc2 in range(2):
                            db = d0 - 512 * qc2
                            if db in DBASES:
                                j = DBASES.index(db)
                            elif db <= -256:
                                j = 6
                            else:
                                j = 7
                            nc.vector.tensor_mul(
                                at_t[:, qc2, :],
                                raw[:, qc2, :],
                                path[:, hh, j, :],
                            )
                    return at_t

                prev = None
                for kti in range(NKT):
                    d0 = kti * 128 - qcp * 1024
                    uniform = d0 <= -256 or d0 >= 1152
                    while bgq and bgq[0][0] <= kti:
                        bgq.popleft()[1]()
                    sA = pspool.tile([128, 2, QC], F32, tag="sps")
                    emit_scores(sA, 0, kti)
                    if prev is not None:
                        emit_av(0, prev[0][0], prev[1])
                    sB = pspool.tile([128, 2, QC], F32, tag="sps")
                    emit_scores(sB, 1, kti)
                    if prev is not None:
                        emit_av(1, prev[0][1], prev[1])
                    at0 = emit_exp(sA, 0, kti, d0, uniform)
                    at1 = emit_exp(sB, 1, kti, d0, uniform)
                    prev = ((at0, at1), kti)
                emit_av(0, prev[0][0], prev[1])
                emit_av(1, prev[0][1], prev[1])
                # normalize: divide O^T columns by the ones-row denominator
                for i in range(4):
                    hh, qc2 = i // 2, i % 2
                    qc = qcp * 2 + qc2
                    oc = rp.tile([DK + 1, QC], F32, tag="oc")
                    nc.vector.tensor_copy(oc, o_pss[i])
                    den0 = rp.tile([1, QC], F32, tag="dn")
                    nc.vector.tensor_copy(den0, oc[DK : DK + 1, :])
                    r1 = rp.tile([1, QC], F32, tag="r1")
                    nc.vector.reciprocal(r1, den0)
                    rb = rp.tile([64, QC], F32, tag="rb")
                    r1ap = r1[0:1, :]
                    nc.sync.dma_start(
                        out=rb,
                        in_=bass.AP(
                            tensor=r1ap.tensor,
                            offset=r1ap.offset,
                            ap=[
                                [list(r1ap.ap[0])[0], 1],
                                [0, 64],
                                list(r1ap.ap[-1]),
                            ],
                        ),
                    )
                    nc.vector.tensor_mul(
                        ot[hh * 64 : (hh + 1) * 64, m, qc * QC : (qc + 1) * QC],
                        oc[0:DK, :],
                        rb,
                    )

            # ---- emission schedule ----
            # Prefix (before the EXP rhythm starts) may be any parity; the
            # v_half(0,0) at slot-0 of the first window is effectively part
            # of it.  All later insertions are pairs.
            def pair(a, b):
                def emit():
                    a()
                    b()

                return emit

            q_half(0, 0)()
            k_half(0, 0)()
            q_half(0, 1)()
            window(
                0,
                0,
                [
                    (0, v_half(0, 0)),
                    (1, pair(k_half(0, 1), v_half(0, 1))),
                    (2, pair(v_half(0, 2), v_half(0, 3))),
                    (4, pair(v_half(1, 0), v_half(1, 1))),
                    (5, pair(q_half(0, 2), k_half(0, 2))),
                    (6, pair(v_half(1, 2), v_half(1, 3))),
                    (8, pair(v_half(2, 0), v_half(2, 1))),
                    (9, pair(q_half(0, 3), k_half(0, 3))),
                    (10, pair(v_half(2, 2), v_half(2, 3))),
                    (12, pair(v_half(3, 0), v_half(3, 1))),
                    (14, pair(v_half(3, 2), v_half(3, 3))),
                    (15, dma_pack(dma_xq, 1, 0)),
                    (15, dma_pack(dma_xq, 1, 1)),
                    (15, dma_pack(dma_xq, 1, 2)),
                    (15, dma_pack(dma_xq, 1, 3)),
                ],
            )
            window(
                0,
                1,
                [
                    (0, dma_pack(dma_path, 1)),
                    (1, pair(q_half(1, 0), k_half(1, 0))),
                    (4, pair(q_half(1, 1), k_half(1, 1))),
                    (7, pair(q_half(1, 2), k_half(1, 2))),
                    (10, pair(q_half(1, 3), k_half(1, 3))),
                    (13, dma_pack(dma_xq, 2, 0)),
                    (13, dma_pack(dma_xq, 2, 1)),
                    (13, dma_pack(dma_xq, 2, 2)),
                    (13, dma_pack(dma_xq, 2, 3)),
                ],
            )
            window(1, 0, [])
            window(
                1,
                1,
                [
                    (0, dma_pack(dma_path, 2)),
                    (1, pair(q_half(2, 0), k_half(2, 0))),
                    (4, pair(q_half(2, 1), k_half(2, 1))),
                    (7, pair(q_half(2, 2), k_half(2, 2))),
                    (10, pair(q_half(2, 3), k_half(2, 3))),
                    (13, dma_pack(dma_xq, 3, 0)),
                    (13, dma_pack(dma_xq, 3, 1)),
                    (13, dma_pack(dma_xq, 3, 2)),
                    (13, dma_pack(dma_xq, 3, 3)),
                ],
            )
            window(2, 0, [])
            window(
                2,
                1,
                [
                    (0, dma_pack(dma_path, 3)),
                    (1, pair(q_half(3, 0), k_half(3, 0))),
                    (4, pair(q_half(3, 1), k_half(3, 1))),
                    (7, pair(q_half(3, 2), k_half(3, 2))),
                    (10, pair(q_half(3, 3), k_half(3, 3))),
                ],
            )
            window(3, 0, [])
            window(3, 1, [])
            for st_g in range(16):
                out_pack(st_g)()
    nc.compile()
    return nc


def _bias_offsets(rel_bias_table):
    """bias value per relative offset d = k - q in [-2047, 2047] -> [H, 4095].

    Mirrors reference._relative_position_bucket op-for-op in jax so that the
    bucket indices match the grading reference bit-exactly (the jax backend's
    jnp.log is an approximation, so host numpy log can flip int-cast
    boundaries).
    """
    import jax.numpy as jnp

    d = jnp.arange(-(S - 1), S)
    nb = 16
    buckets = (d > 0).astype(jnp.int32) * nb
    rp = jnp.abs(d)
    max_exact = nb // 2
    is_small = rp < max_exact
    rl = max_exact + (
        jnp.log(jnp.maximum(rp, 1).astype(jnp.float32) / max_exact)
        / math.log(128 / max_exact)
        * (nb - max_exact)
    ).astype(jnp.int32)
    rl = jnp.minimum(rl, nb - 1)
    bucket = np.asarray(buckets + jnp.where(is_small, rp, rl))  # [4095]
    return np.asarray(rel_bias_table)[bucket, :].T.astype(np.float32)  # [H, 4095]


def kernel(hidden_states, Wq, Wk, Wv, Wo, rel_bias_table, _trace=False):
    hidden_states = np.ascontiguousarray(hidden_states, dtype=np.float32)
    Wq = np.asarray(Wq, dtype=np.float32)
    Wk = np.asarray(Wk, dtype=np.float32)
    Wv = np.asarray(Wv, dtype=np.float32)
    Wo = np.asarray(Wo, dtype=np.float32)
    rel_bias_table = np.asarray(rel_bias_table, dtype=np.float32)

    if "nc" not in _NC_CACHE:
        _NC_CACHE["nc"] = _build_nc()
    nc = _NC_CACHE["nc"]

    bias_off = _bias_offsets(rel_bias_table)  # [H, 4095]
    # patterns[g][h, j, p, c] = exp(bias(d = DBASES[j] + p - c)) for head
    # g*8+h; multiplicative (applied after exp on the scalar engine).
    pidx = (
        np.array(DBASES)[None, :, None, None]
        + np.arange(128)[None, None, :, None]
        - np.arange(QC)[None, None, None, :]
        + (S - 1)
    )  # [1, 6, 128, 512]
    in_maps = []
    for core in range(8):
        b, g = core // 2, core % 2
        heads = slice(g * HG, (g + 1) * HG)
        pat6 = bias_off[heads][
            np.arange(HG)[:, None, None, None], pidx
        ]  # [8, 6, 128, 512]
        pat = np.zeros((HG, 8, 128, QC), dtype=np.float32)
        pat[:, :6] = pat6
        for h in range(HG):
            pat[h, 6] = rel_bias_table[15, g * HG + h]
            pat[h, 7] = rel_bias_table[31, g * HG + h]
        pat = np.exp(pat)
        cst = np.zeros((128, 2 * HG), dtype=np.float32)
        for h in range(HG):
            cst[:, 2 * h] = rel_bias_table[15, g * HG + h]  # far-left bucket
            cst[:, 2 * h + 1] = rel_bias_table[31, g * HG + h]  # far-right bucket
        in_maps.append(
            {
                "xT": np.ascontiguousarray(hidden_states[b].T).astype(MMNP),
                "wq": np.ascontiguousarray(Wq[:, g * HDG : (g + 1) * HDG]).astype(MMNP),
                "wk": np.ascontiguousarray(Wk[:, g * HDG : (g + 1) * HDG]).astype(MMNP),
                "wv": np.ascontiguousarray(Wv[:, g * HDG : (g + 1) * HDG]).astype(MMNP),
                "wo": np.ascontiguousarray(Wo[g * HDG : (g + 1) * HDG, :]).astype(MMNP),
                "pat": np.ascontiguousarray(pat.astype(ml_dtypes.bfloat16)),
                "cst": cst,
            }
        )

    res = run_bass_kernel_spmd(nc, in_maps, core_ids=list(range(8)), trace=_trace)
    global LAST_RESULTS
    LAST_RESULTS = res
    out = np.empty((B, S, D), dtype=np.float32)
    for b in range(B):
        out[b] = res.results[2 * b]["out"] + res.results[2 * b + 1]["out"]
    return out


LAST_RESULTS = None


# revision 17
# speedup vs baseline: 1.2382x; 1.2382x over previous
"""T5-style attention layer (B=4, S=2048, D=1024, H=16, DK=64) on 8 trn2 cores.

Sharding: batch (4) x head-group (2 groups of 8 heads). Core c -> batch c//2,
head-group c%2. Each core computes its batch's attention output restricted to
its 8 heads, projected through its Wo row-slice -> partial [S, D] output.
Host sums the two head-group partials per batch (the "all-reduce").

v2 pipeline (matmuls bf16, fp32 PSUM):
  The ScalarE exp stream is the hard floor (~2.3us per k-tile, 128 k-tile
  instances per core); everything else is scheduled into its shadow.
  - Scores S^T[k,q] per (head-pair m, 1024-q chunk): two [128,2,512] PSUM
    tiles (one per head), row-split matmul pairs. exp on ScalarE reads each
    head's tile; the two heads alternate as natural double buffering.
  - T5 relative-position bias: multiplicative exp(bias) Toeplitz patterns
    applied by DVE (bf16, 4 elem/cycle) AFTER exp on near-diagonal tiles;
    far-saturated tiles fold the per-head constant into the exp bias operand.
  - AV: O^T[dk+1, q] += [V|1].T @ at, one k-tile behind scores; row 64 is
    the softmax denominator. Normalize: reciprocal_approx_fast (DVE) +
    GpSimd partition_broadcast + DVE mul.
  - Q/K/V projections and the output projection are emitted as "background
    packs" interleaved into the k-tile stream so the PE works under the
    ScalarE shadow; only the second half of the output projection trails.
  PSUM: 2 score tiles (2 banks each) rotating in a shared pool (also used
  by projection/output packs) + 4 AV accumulators = 8 banks exactly.
"""

import math
from collections import deque

import ml_dtypes
import numpy as np

import concourse.bass as bass
import concourse.mybir as mybir
import concourse.tile as tile
from concourse import bacc
from concourse.bass_utils import run_bass_kernel_spmd

F32 = mybir.dt.float32
BF16 = mybir.dt.bfloat16
MMDT = BF16
MMNP = ml_dtypes.bfloat16
AF = mybir.ActivationFunctionType

B, S, D, H, DK = 4, 2048, 1024, 16, 64
HG = 8  # heads per core
HDG = HG * DK  # 512
QC = 512  # q chunk width
NKT = S // 128  # 16 k tiles
DBASES = [-128, 0, 128, 256, 384, 512]  # near-band k0-q0 alignments

_NC_CACHE = {}


def _build_nc():
    nc = bacc.Bacc(None, target_bir_lowering=False, debug=False)
    xT = nc.dram_tensor("xT", [D, S], MMDT, kind="ExternalInput")
    wq = nc.dram_tensor("wq", [D, HDG], MMDT, kind="ExternalInput")
    wk = nc.dram_tensor("wk", [D, HDG], MMDT, kind="ExternalInput")
    wv = nc.dram_tensor("wv", [D, HDG], MMDT, kind="ExternalInput")
    wo = nc.dram_tensor("wo", [HDG, D], MMDT, kind="ExternalInput")
    pat = nc.dram_tensor("pat", [HG, len(DBASES) + 2, 128, QC], BF16, kind="ExternalInput")
    cst = nc.dram_tensor("cst", [128, 2 * HG], F32, kind="ExternalInput")
    outd = nc.dram_tensor("out", [S, D], F32, kind="ExternalOutput")

    with tile.TileContext(nc) as tc:
        with tc.tile_pool(name="persist", bufs=1) as persist, tc.tile_pool(
            name="xp", bufs=4
        ) as xpool, tc.tile_pool(name="patp", bufs=2) as patp, tc.tile_pool(
            name="attnp", bufs=4
        ) as attnp, tc.tile_pool(name="rp", bufs=2) as rp, tc.tile_pool(
            name="obp", bufs=2
        ) as obp, tc.tile_pool(name="ps", bufs=2, space="PSUM") as pspool, tc.tile_pool(
            name="ps_o", bufs=4, space="PSUM"
        ) as opool:
            qt = persist.tile([128, 4, S], MMDT, tag="qt")
            kt = persist.tile([128, 4, S], MMDT, tag="kt")
            vt = persist.tile([128, NKT, HG, DK + 1], MMDT, tag="vt")
            csts = persist.tile([128, 2 * HG], F32, tag="csts")
            ot = persist.tile([128, 4, S], MMDT, tag="ot")
            wos = persist.tile([128, 4, D], MMDT, tag="wos")
            wqs = persist.tile([128, 8, HDG], MMDT, tag="wqs")
            wks = persist.tile([128, 8, HDG], MMDT, tag="wks")
            wvs = persist.tile([128, 8, HDG], MMDT, tag="wvs")

            # ---- initial DMAs (ordered so the first QK pack unblocks asap) ----
            nc.sync.dma_start(out=wqs, in_=wq.rearrange("(dc p) n -> p dc n", p=128))
            xqs = {}

            def dma_xq(m, sc):
                t = xpool.tile([128, 8, QC], MMDT, tag="xq")
                nc.sync.dma_start(
                    out=t,
                    in_=xT[:, sc * QC : (sc + 1) * QC].rearrange(
                        "(dc p) s -> p dc s", p=128
                    ),
                )
                xqs[(m, sc)] = t

            dma_xq(0, 0)
            nc.sync.dma_start(out=wks, in_=wk.rearrange("(dc p) n -> p dc n", p=128))
            dma_xq(0, 1)
            nc.sync.dma_start(out=wvs, in_=wv.rearrange("(dc p) n -> p dc n", p=128))
            nc.sync.dma_start(out=csts, in_=cst[:, :])
            nc.sync.dma_start(out=wos, in_=wo.rearrange("(m p) n -> p m n", p=128))
            dma_xq(0, 2)
            dma_xq(0, 3)
            paths = {}

            def dma_path(m):
                t = patp.tile([128, 2, len(DBASES) + 2, QC], BF16, tag="pth")
                nc.sync.dma_start(
                    out=t,
                    in_=pat[2 * m : 2 * m + 2].rearrange("h j p c -> p h j c"),
                )
                paths[m] = t

            dma_path(0)

            # ---- background pack emitters ----
            # Each "half" allocates ONE tile from the shared PSUM pool.  In
            # the steady state the pool alternates between the two score
            # tiles (h0/h1), so background halves must be emitted in PAIRS
            # to preserve slot parity — a lone insertion permanently
            # misaligns the scores->EXP WAR chain and costs ~0.6us of
            # ScalarE idle per k-tile afterwards.
            def q_half(m, sc):
                def emit():
                    ps = pspool.tile([128, 2, QC], F32, tag="sps")
                    xq = xqs[(m, sc)]
                    for dc in range(8):
                        nc.tensor.matmul(
                            ps[:, 0, :],
                            wqs[:, dc, m * 128 : (m + 1) * 128],
                            xq[:, dc, :],
                            start=(dc == 0),
                            stop=(dc == 7),
                        )
                    nc.vector.tensor_copy(
                        qt[:, m, sc * QC : (sc + 1) * QC], ps[:, 0, :]
                    )

                return emit

            def k_half(m, sc):
                def emit():
                    ps = pspool.tile([128, 2, QC], F32, tag="sps")
                    xq = xqs[(m, sc)]
                    for dc in range(8):
                        nc.tensor.matmul(
                            ps[:, 0, :],
                            wks[:, dc, m * 128 : (m + 1) * 128],
                            xq[:, dc, :],
                            start=(dc == 0),
                            stop=(dc == 7),
                        )
                    nc.vector.tensor_copy(
                        kt[:, m, sc * QC : (sc + 1) * QC], ps[:, 0, :]
                    )

                return emit

            def v_half(sc, st):
                def emit():
                    ps = pspool.tile([128, 2, QC], F32, tag="sps")
                    xq = xqs[(0, sc)]
                    for dc in range(8):
                        nc.tensor.matmul(
                            ps[:, 0, :],
                            xq[:, dc, st * 128 : (st + 1) * 128],
                            wvs[:, dc, :],
                            start=(dc == 0),
                            stop=(dc == 7),
                        )
                    nc.vector.tensor_copy(
                        vt[:, sc * 4 + st, :, 0:DK],
                        ps[:, 0, :].rearrange("p (h d) -> p h d", h=HG),
                    )
                    # ones column for the softmax denominator, written by the
                    # same engine as the V copy so ordering vs AV is tracked
                    nc.vector.memset(vt[:, sc * 4 + st, :, DK : DK + 1], 1.0)

                return emit

            def out_pack(st_g):
                def emit():
                    ps = pspool.tile([128, 2, QC], F32, tag="sps")
                    for nck in range(2):
                        for m2 in range(4):
                            nc.tensor.matmul(
                                ps[:, nck, :],
                                ot[:, m2, st_g * 128 : (st_g + 1) * 128],
                                wos[:, m2, nck * 512 : (nck + 1) * 512],
                                start=(m2 == 0),
                                stop=(m2 == 3),
                            )
                    ob = obp.tile([128, 2, QC], F32, tag="ob")
                    nc.vector.tensor_copy(ob, ps)
                    nc.sync.dma_start(
                        out=outd[st_g * 128 : (st_g + 1) * 128, :].rearrange(
                            "p (a b) -> p a b", a=2
                        ),
                        in_=ob,
                    )

                return emit

            def dma_pack(fn, *args):
                def emit():
                    fn(*args)

                return emit

            # ---- attention window ----
            def window(m, qcp, bg):
                bgq = deque(sorted(bg, key=lambda kv: kv[0]))
                path = paths[m]
                o_pss = [
                    opool.tile([DK + 1, QC], F32, tag="ops", name=f"o{i}")
                    for i in range(4)
                ]
                # Emission order per k-tile keeps ScalarE gapless:
                # [pack pair?][scores h0][AV h0(prev)][scores h1][AV h1(prev)]
                # then EXP h0 + bias mults, EXP h1 + bias mults.  scores-h0 of
                # tile t+1 only WARs on EXP-h0(t), which ends one EXP before
                # the ACT queue frees, so the next EXP's input is always ready.
                def emit_scores(sp, hh, kti):
                    for qc2 in range(2):
                        qc = qcp * 2 + qc2
                        nc.tensor.matmul(
                            sp[:, qc2, :],
                            kt[
                                hh * 64 : (hh + 1) * 64,
                                m,
                                kti * 128 : (kti + 1) * 128,
                            ],
                            qt[
                                hh * 64 : (hh + 1) * 64,
                                m,
                                qc * QC : (qc + 1) * QC,
                            ],
                            start=True,
                            stop=True,
                        )

                def emit_av(hh, at_prev, pkti):
                    for qc2 in range(2):
                        nc.tensor.matmul(
                            o_pss[hh * 2 + qc2],
                            vt[:, pkti, 2 * m + hh, :],
                            at_prev[:, qc2, :],
                            start=(pkti == 0),
                            stop=(pkti == NKT - 1),
                        )

                def emit_exp(sp, hh, kti, d0, uniform):
                    h = 2 * m + hh
                    at_t = attnp.tile([128, 2, QC], MMDT, tag="at")
                    if uniform:
                        col = 2 * h + (0 if d0 <= -256 else 1)
                        nc.scalar.activation(
                            at_t, sp, AF.Exp, bias=csts[:, col : col + 1]
                        )
                    else:
                        raw = attnp.tile([128, 2, QC], MMDT, tag="raw", bufs=2)
                        nc.scalar.activation(raw, sp, AF.Exp)
                        for qc2 in range(2):
                            db = d0 - 512 * qc2
                            if db in DBASES:
                                j = DBASES.index(db)
                            elif db <= -256:
                                j = 6
                            else:
                                j = 7
                            nc.vector.tensor_mul(
                                at_t[:, qc2, :],
                                raw[:, qc2, :],
                                path[:, hh, j, :],
                            )
                    return at_t

                prev = None
                for kti in range(NKT):
                    d0 = kti * 128 - qcp * 1024
                    uniform = d0 <= -256 or d0 >= 1152
                    while bgq and bgq[0][0] <= kti:
                        bgq.popleft()[1]()
                    sA = pspool.tile([128, 2, QC], F32, tag="sps")
                    emit_scores(sA, 0, kti)
                    if prev is not None:
                        emit_av(0, prev[0][0], prev[1])
                    sB = pspool.tile([128, 2, QC], F32, tag="sps")
                    emit_scores(sB, 1, kti)
                    if prev is not None:
                        emit_av(1, prev[0][1], prev[1])
                    at0 = emit_exp(sA, 0, kti, d0, uniform)
                    at1 = emit_exp(sB, 1, kti, d0, uniform)
                    prev = ((at0, at1), kti)
                emit_av(0, prev[0][0], prev[1])
                emit_av(1, prev[0][1], prev[1])
                # Normalize, phase 1 (immediate): copy O^T off PSUM (frees the
                # AV banks for the next window) and pack the four denominator
                # rows at partitions 0/32/64/96 of one tile.  Phase 2 (the
                # slow reciprocal, broadcast and muls) is DEFERRED into the
                # next window's k-tile slots so the DVE FIFO never carries a
                # multi-us burst that would stall the next window's bias
                # multiplies (and through the at-tile WAR chain, the EXPs).
                ocs = []
                bn = rp.tile([97, QC], F32, tag="bn", bufs=1)
                for i in range(4):
                    oc = rp.tile([DK + 1, QC], F32, tag="oc", bufs=4)
                    nc.vector.tensor_copy(oc, o_pss[i])
                    nc.vector.tensor_copy(
                        bn[32 * i : 32 * i + 1, :], oc[DK : DK + 1, :]
                    )
                    ocs.append(oc)

                def finish_recip():
                    bnr = rp.tile([97, QC], F32, tag="bnr", bufs=1)
                    nc.vector.reciprocal(bnr, bn)
                    finish_recip.bnr = bnr

                def finish_slice(i):
                    def emit():
                        hh, qc2 = i // 2, i % 2
                        qc = qcp * 2 + qc2
                        bnr = finish_recip.bnr
                        rb = rp.tile([64, QC], F32, tag="rb")
                        rap = bnr[32 * i : 32 * i + 1, :]
                        nc.sync.dma_start(
                            out=rb,
                            in_=bass.AP(
                                tensor=rap.tensor,
                                offset=rap.offset,
                                ap=[
                                    [list(rap.ap[0])[0], 1],
                                    [0, 64],
                                    list(rap.ap[-1]),
                                ],
                            ),
                        )
                        nc.vector.tensor_mul(
                            ot[
                                hh * 64 : (hh + 1) * 64, m, qc * QC : (qc + 1) * QC
                            ],
                            ocs[i][0:DK, :],
                            rb,
                        )

                    return emit

                return [finish_recip] + [finish_slice(i) for i in range(4)]

            # ---- emission schedule ----
            # Prefix (before the EXP rhythm starts) may be any parity; the
            # v_half(0,0) at slot-0 of the first window is effectively part
            # of it.  All later insertions are pairs.
            def pair(a, b):
                def emit():
                    a()
                    b()

                return emit

            q_half(0, 0)()
            k_half(0, 0)()
            q_half(0, 1)()
            dfr = window(
                0,
                0,
                [
                    (0, v_half(0, 0)),
                    (1, pair(k_half(0, 1), v_half(0, 1))),
                    (2, pair(v_half(0, 2), v_half(0, 3))),
                    (4, pair(v_half(1, 0), v_half(1, 1))),
                    (5, pair(q_half(0, 2), k_half(0, 2))),
                    (6, pair(v_half(1, 2), v_half(1, 3))),
                    (8, pair(v_half(2, 0), v_half(2, 1))),
                    (9, pair(q_half(0, 3), k_half(0, 3))),
                    (10, pair(v_half(2, 2), v_half(2, 3))),
                    (12, pair(v_half(3, 0), v_half(3, 1))),
                    (14, pair(v_half(3, 2), v_half(3, 3))),
                    (15, dma_pack(dma_xq, 1, 0)),
                    (15, dma_pack(dma_xq, 1, 1)),
                    (15, dma_pack(dma_xq, 1, 2)),
                    (15, dma_pack(dma_xq, 1, 3)),
                ],
            )
            dfr = window(
                0,
                1,
                [(1 + i, f) for i, f in enumerate(dfr)]
                + [
                    (0, dma_pack(dma_path, 1)),
                    (1, pair(q_half(1, 0), k_half(1, 0))),
                    (4, pair(q_half(1, 1), k_half(1, 1))),
                    (7, pair(q_half(1, 2), k_half(1, 2))),
                    (10, pair(q_half(1, 3), k_half(1, 3))),
                    (13, dma_pack(dma_xq, 2, 0)),
                    (13, dma_pack(dma_xq, 2, 1)),
                    (13, dma_pack(dma_xq, 2, 2)),
                    (13, dma_pack(dma_xq, 2, 3)),
                ],
            )
            dfr = window(1, 0, [(1 + i, f) for i, f in enumerate(dfr)])
            dfr = window(
                1,
                1,
                [(1 + i, f) for i, f in enumerate(dfr)]
                + [
                    (0, dma_pack(dma_path, 2)),
                    (1, pair(q_half(2, 0), k_half(2, 0))),
                    (4, pair(q_half(2, 1), k_half(2, 1))),
                    (7, pair(q_half(2, 2), k_half(2, 2))),
                    (10, pair(q_half(2, 3), k_half(2, 3))),
                    (13, dma_pack(dma_xq, 3, 0)),
                    (13, dma_pack(dma_xq, 3, 1)),
                    (13, dma_pack(dma_xq, 3, 2)),
                    (13, dma_pack(dma_xq, 3, 3)),
                ],
            )
            dfr = window(2, 0, [(1 + i, f) for i, f in enumerate(dfr)])
            dfr = window(
                2,
                1,
                [(1 + i, f) for i, f in enumerate(dfr)]
                + [
                    (0, dma_pack(dma_path, 3)),
                    (1, pair(q_half(3, 0), k_half(3, 0))),
                    (4, pair(q_half(3, 1), k_half(3, 1))),
                    (7, pair(q_half(3, 2), k_half(3, 2))),
                    (10, pair(q_half(3, 3), k_half(3, 3))),
                ],
            )
            dfr = window(3, 0, [(1 + i, f) for i, f in enumerate(dfr)])
            # (3,0)'s deferred normalize finishes at slots 1-5; the first half
            # of the output projection (s < 1024, all ot ready) rides along as
            # parity-preserving pairs.
            dfr = window(
                3,
                1,
                [(1 + i, f) for i, f in enumerate(dfr)]
                + [
                    (6, pair(out_pack(0), out_pack(1))),
                    (8, pair(out_pack(2), out_pack(3))),
                    (10, pair(out_pack(4), out_pack(5))),
                    (12, pair(out_pack(6), out_pack(7))),
                ],
            )
            # tail: finish (3,1)'s normalize interleaved with the second half
            # of the output projection (qc2=0 slices feed out rows 1024-1535)
            dfr[0]()  # reciprocal
            dfr[1]()  # slice (h0, qc2=0)
            dfr[3]()  # slice (h1, qc2=0)
            for st_g in range(8, 12):
                out_pack(st_g)()
            dfr[2]()  # slice (h0, qc2=1)
            dfr[4]()  # slice (h1, qc2=1)
            for st_g in range(12, 16):
                out_pack(st_g)()
    nc.compile()
    return nc


def _bias_offsets(rel_bias_table):
    """bias value per relative offset d = k - q in [-2047, 2047] -> [H, 4095].

    Mirrors reference._relative_position_bucket op-for-op in jax so that the
    bucket indices match the grading reference bit-exactly (the jax backend's
    jnp.log is an approximation, so host numpy log can flip int-cast
    boundaries).
    """
    import jax.numpy as jnp

    d = jnp.arange(-(S - 1), S)
    nb = 16
    buckets = (d > 0).astype(jnp.int32) * nb
    rp = jnp.abs(d)
    max_exact = nb // 2
    is_small = rp < max_exact
    rl = max_exact + (
        jnp.log(jnp.maximum(rp, 1).astype(jnp.float32) / max_exact)
        / math.log(128 / max_exact)
        * (nb - max_exact)
    ).astype(jnp.int32)
    rl = jnp.minimum(rl, nb - 1)
    bucket = np.asarray(buckets + jnp.where(is_small, rp, rl))  # [4095]
    return np.asarray(rel_bias_table)[bucket, :].T.astype(np.float32)  # [H, 4095]


def kernel(hidden_states, Wq, Wk, Wv, Wo, rel_bias_table, _trace=False):
    hidden_states = np.ascontiguousarray(hidden_states, dtype=np.float32)
    Wq = np.asarray(Wq, dtype=np.float32)
    Wk = np.asarray(Wk, dtype=np.float32)
    Wv = np.asarray(Wv, dtype=np.float32)
    Wo = np.asarray(Wo, dtype=np.float32)
    rel_bias_table = np.asarray(rel_bias_table, dtype=np.float32)

    if "nc" not in _NC_CACHE:
        _NC_CACHE["nc"] = _build_nc()
    nc = _NC_CACHE["nc"]

    bias_off = _bias_offsets(rel_bias_table)  # [H, 4095]
    # patterns[g][h, j, p, c] = exp(bias(d = DBASES[j] + p - c)) for head
    # g*8+h; multiplicative (applied after exp on the scalar engine).
    pidx = (
        np.array(DBASES)[None, :, None, None]
        + np.arange(128)[None, None, :, None]
        - np.arange(QC)[None, None, None, :]
        + (S - 1)
    )  # [1, 6, 128, 512]
    in_maps = []
    for core in range(8):
        b, g = core // 2, core % 2
        heads = slice(g * HG, (g + 1) * HG)
        pat6 = bias_off[heads][
            np.arange(HG)[:, None, None, None], pidx
        ]  # [8, 6, 128, 512]
        pat = np.zeros((HG, 8, 128, QC), dtype=np.float32)
        pat[:, :6] = pat6
        for h in range(HG):
            pat[h, 6] = rel_bias_table[15, g * HG + h]
            pat[h, 7] = rel_bias_table[31, g * HG + h]
        pat = np.exp(pat)
        cst = np.zeros((128, 2 * HG), dtype=np.float32)
        for h in range(HG):
            cst[:, 2 * h] = rel_bias_table[15, g * HG + h]  # far-left bucket
            cst[:, 2 * h + 1] = rel_bias_table[31, g * HG + h]  # far-right bucket
        in_maps.append(
            {
                "xT": np.ascontiguousarray(hidden_states[b].T).astype(MMNP),
                "wq": np.ascontiguousarray(Wq[:, g * HDG : (g + 1) * HDG]).astype(MMNP),
                "wk": np.ascontiguousarray(Wk[:, g * HDG : (g + 1) * HDG]).astype(MMNP),
                "wv": np.ascontiguousarray(Wv[:, g * HDG : (g + 1) * HDG]).astype(MMNP),
                "wo": np.ascontiguousarray(Wo[g * HDG : (g + 1) * HDG, :]).astype(MMNP),
                "pat": np.ascontiguousarray(pat.astype(ml_dtypes.bfloat16)),
                "cst": cst,
            }
        )

    res = run_bass_kernel_spmd(nc, in_maps, core_ids=list(range(8)), trace=_trace)
    global LAST_RESULTS
    LAST_RESULTS = res
    out = np.empty((B, S, D), dtype=np.float32)
    for b in range(B):
        out[b] = res.results[2 * b]["out"] + res.results[2 * b + 1]["out"]
    return out


LAST_RESULTS = None
